# revision 30
# baseline (speedup 1.0000x reference)
# Trainium2 Bass kernel for nn_CrossFrequencyInteraction.
#
# Reference computation (per batch item, two symmetric branches):
#   q = Wq @ x_q;  k = Wk @ x_kv;  v = Wv @ x_kv          (1x1 convs, C=256)
#   out = softmax_n(q) applied against ctx = softmax_n(k) @ v^T  (linear attn)
#   inter = Wp @ out;  x_q += inter
#   then training-mode BatchNorm over (B,H,W) on both updated rgb tensors.
#
# Sharding: data-parallel over batch (B=8 -> 1 item per core, 8 cores).
#
# Numerics / structure (validated against the fp32 reference, absmax-rel
# ~3e-3 vs the 2e-2 gate):
#   - The attention path contributes |inter| <~ 1e-4 against |x| ~ 5, so it
#     runs end-to-end in fp8 e4m3 with DoubleRow matmuls (K=256 per
#     instruction, 0.5 cyc/col).  Host uploads x/8 and weights*8 so q,k,v
#     keep their exact scale going into exp while both operands sit in fp8
#     normal range.
#   - |inter|'s contribution to the BN batch statistics is ~2e-4 relative,
#     far below tolerance, so BN stats are computed on the host from x
#     alone (full batch is available host-side).  b_proj shifts the mean
#     exactly and is folded into the host-side mean.  The BN affine then
#     folds into the upload (xqb' = x*g/sd + t) and into the inter
#     eviction scale, fusing residual + BN into one scalar_tensor_tensor
#     per chunk.  No collective, no bn_stats, no separate normalize pass.
#   - b_q, b_k shift softmax inputs by a per-row constant -> cancel exactly.
#   - b_v folded into ctx at eviction (exact when sum softmax_k = 1).
#   - softmax denominators (k and q) are folded into the tiny
#     M = Wp.blockdiag(ctx^T) matrix (computed via PE transpose of ctx),
#     so attention-out + projection become a single fp8 [256,256] @
#     [256,4096] DoubleRow matmul per branch.  M underflows fp8 and is
#     scaled by 2^22, descaled for free in the eviction STT.
#   - kT/vT are produced in transposed layout by using x as the stationary
#     matmul operand; softmax-k denominators come free from a ones column
#     appended to the vT stream tiles.
#
# Scheduling: engine queues execute in issue order, so branch-0 Q-conv
# chunks are interleaved with branch-1 KV pairs (and branch-0 inter with
# branch-1 Q) to keep the PE dense while psum evictions drain; ctx matmuls
# run one pair behind their evictions.

import numpy as np

C = 256
N = 4096
P = 128
NTP = 16          # pairs of 128-wide n-tiles (KV phase, fp8-DR over pairs)
NCORES = 8
HD = 64
EPS = 1e-5
SW = 8.0          # host weight/input scale around fp8
SMT = float(2.0 ** 22)   # fp8 scale for M^T
ISMT = float(2.0 ** -22)

_CACHE = {}


def _build():
    import concourse.bass as bass
    import concourse.bacc as bacc
    import concourse.tile as tile
    from concourse import mybir
    from contextlib import ExitStack

    F32 = mybir.dt.float32
    BF16 = mybir.dt.bfloat16
    F8 = mybir.dt.float8e4
    OP = mybir.AluOpType
    AF = mybir.ActivationFunctionType
    AX = mybir.AxisListType
    DR = mybir.MatmulPerfMode.DoubleRow

    nc = bacc.Bacc("TRN2", num_devices=NCORES)

    xq8_d = [nc.dram_tensor(n_, [P, 2, N], F8, kind="ExternalInput")
             for n_ in ("xq8_1", "xq8_2")]
    xkv8_d = [nc.dram_tensor(n_, [P, 2, N], F8, kind="ExternalInput")
              for n_ in ("xkv8_1", "xkv8_2")]
    # pre-normalized residual base: x*g/sd + (beta - (mu+b_proj)*g/sd)
    xqb_d = [nc.dram_tensor(n_, [C, N], BF16, kind="ExternalInput")
             for n_ in ("xqb_1", "xqb_2")]
    # wt8: [128,2,1024]: per branch 512 cols of [Wk^T|Wv^T]*8, DR-interleaved
    wt8_d = nc.dram_tensor("wt8", [P, 2, 1024], F8, kind="ExternalInput")
    # wq8: [128,2,512]: per branch 256 cols of Wq^T*8, DR-interleaved
    wq8_d = nc.dram_tensor("wq8", [P, 2, 512], F8, kind="ExternalInput")
    # wpt: [128, 4*256] bf16; block (2b+p) = Wp_b^T[p*128:(p+1)*128, :]
    wpt_d = nc.dram_tensor("wpt", [P, 1024], BF16, kind="ExternalInput")
    id_d = nc.dram_tensor("id128", [P, P], BF16, kind="ExternalInput")
    # bp: [128, 8]: (spi b0k0, b0k1, b1k0, b1k1, bv b0p0, b0p1, b1p0, b1p1)
    bp_d = nc.dram_tensor("bp", [P, 8], F32, kind="ExternalInput")
    # residual base pre-scaled by 1/spi (for the PE-accumulated eviction
    # path: psum += xqb2, evicted by the scalar engine * spi)
    xqb2_d = [nc.dram_tensor(n_, [C, N], BF16, kind="ExternalInput")
              for n_ in ("xqb2_1", "xqb2_2")]
    out_d = [nc.dram_tensor(n_, [C, N], F32, kind="ExternalOutput")
             for n_ in ("out1", "out2")]

    with ExitStack() as ctx:
        tc = ctx.enter_context(tile.TileContext(nc))
        const = ctx.enter_context(tc.tile_pool(name="const", bufs=1))
        xp = ctx.enter_context(tc.tile_pool(name="xp", bufs=1))
        eqp = ctx.enter_context(tc.tile_pool(name="eqp", bufs=2))
        ekp = ctx.enter_context(tc.tile_pool(name="ekp", bufs=4))
        misc = ctx.enter_context(tc.tile_pool(name="misc", bufs=2))
        stgp = ctx.enter_context(tc.tile_pool(name="stgp", bufs=6))
        psr = ctx.enter_context(tc.tile_pool(name="psr", bufs=3, space="PSUM"))
        qp = ctx.enter_context(tc.tile_pool(name="qp", bufs=1, space="PSUM"))
        ctxp = ctx.enter_context(tc.tile_pool(name="ctxp", bufs=1, space="PSUM"))
        tinyp = ctx.enter_context(tc.tile_pool(name="tinyp", bufs=2, space="PSUM"))

        # ---- loads: critical pieces first on sync; the rest on scalar ----
        wt8 = const.tile([P, 2, 1024], F8, name="wt8", tag="wt8")
        wq8 = const.tile([P, 2, 512], F8, name="wq8", tag="wq8")
        wpt = const.tile([P, 1024], BF16, name="wpt", tag="wpt")
        id_sb = const.tile([P, P], BF16, name="id", tag="id")
        bp_sb = const.tile([P, 8], F32, name="bp", tag="bp")
        xkv8 = [xp.tile([P, 2, N], F8, name=f"xkv8_{b}", tag=f"xkv8_{b}")
                for b in range(2)]
        xq8 = [xp.tile([P, 2, N], F8, name=f"xq8_{b}", tag=f"xq8_{b}")
               for b in range(2)]
        xqb = [[xp.tile([P, N], BF16, name=f"xqb{k}_{b}", tag=f"xqb{k}_{b}")
                for k in range(2)] for b in range(2)]

        nc.sync.dma_start(out=wt8, in_=wt8_d[:, :, :])
        nc.sync.dma_start(out=xkv8[0][:, :, 0:512], in_=xkv8_d[0][:, :, 0:512])
        nc.sync.dma_start(out=wq8, in_=wq8_d[:, :, :])
        for c0, c1 in ((512, 1536), (1536, 2816), (2816, N)):
            nc.sync.dma_start(out=xkv8[0][:, :, c0:c1],
                              in_=xkv8_d[0][:, :, c0:c1])
        for c0, c1 in ((0, 2048), (2048, N)):
            nc.sync.dma_start(out=xq8[0][:, :, c0:c1],
                              in_=xq8_d[0][:, :, c0:c1])
        for c0, c1 in ((0, 1024), (1024, 2048), (2048, N)):
            nc.sync.dma_start(out=xkv8[1][:, :, c0:c1],
                              in_=xkv8_d[1][:, :, c0:c1])
        for c0, c1 in ((0, 2048), (2048, N)):
            nc.sync.dma_start(out=xq8[1][:, :, c0:c1],
                              in_=xq8_d[1][:, :, c0:c1])
        nc.scalar.dma_start(out=wpt, in_=wpt_d[:, :])
        nc.scalar.dma_start(out=id_sb, in_=id_d[:, :])
        nc.scalar.dma_start(out=bp_sb, in_=bp_d[:, :])
        for b in range(2):
            for k in range(2):
                nc.sync.dma_start(out=xqb[b][k],
                                  in_=xqb_d[b][k * P:(k + 1) * P, :])
        xqb2 = [[xp.tile([P, N], BF16, name=f"xqb2_{k}_{b}",
                         tag=f"xqb2_{k}_{b}") for k in range(2)]
                for b in range(2)]
        for b in range(2):
            for k in range(2):
                nc.sync.dma_start(out=xqb2[b][k],
                                  in_=xqb2_d[b][k * P:(k + 1) * P, :])
        # vT ring buffers [128, tt, pair, 129]; col 128 is the ones column
        # that yields softmax-k denominators inside the ctx matmuls.
        vtb = []
        for i in range(3):
            t = const.tile([P, 2, 2, 129], F8, name=f"vtb{i}", tag=f"vtb{i}")
            nc.vector.memset(t[:, :, :, 128], 1.0)
            vtb.append(t)

        st = {0: {}, 1: {}}

        # ---- emission helpers (issue order == engine execution order) ----

        def emit_kv_pair(b, tp):
            # K conv, V conv (fp8-DR) for n-tiles (2tp, 2tp+1) + evictions
            w0 = b * 512
            pk = psr.tile([P, 512], F32, name=f"pk_{b}_{tp}", tag="ps")
            pv = psr.tile([P, 512], F32, name=f"pv_{b}_{tp}", tag="ps")
            for tt in range(2):
                s128 = slice(tp * 256 + tt * P, tp * 256 + (tt + 1) * P)
                nc.tensor.matmul(
                    pk[:, tt * 256:(tt + 1) * 256],
                    lhsT=xkv8[b][:, :, s128],
                    rhs=wt8[:, :, w0:w0 + 256],
                    start=True, stop=True, perf_mode=DR,
                )
            for tt in range(2):
                s128 = slice(tp * 256 + tt * P, tp * 256 + (tt + 1) * P)
                nc.tensor.matmul(
                    pv[:, tt * 256:(tt + 1) * 256],
                    lhsT=xkv8[b][:, :, s128],
                    rhs=wt8[:, :, w0 + 256:w0 + 512],
                    start=True, stop=True, perf_mode=DR,
                )
            ek = ekp.tile([P, 2, 256], F8, name=f"ek_{b}_{tp}", tag="ek")
            nc.scalar.activation(ek.rearrange("p s c -> p (s c)"), pk, AF.Exp)
            vb = vtb[tp % 3]
            nc.vector.tensor_copy(
                vb[:, :, :, 0:P],
                pv.rearrange("p (s g c) -> p s g c", s=2, g=2))
            st[b][f"ek{tp}"] = ek
            st[b][f"vb{tp}"] = vb

        def emit_ctx(b, tp):
            # fp8-DR ctx matmuls for pair tp (issued one pair behind)
            pctx = st[b].get("pctx")
            if pctx is None:
                pctx = ctxp.tile([P, 2, 129], F32, name=f"pctx_{b}", tag="pctx")
                st[b]["pctx"] = pctx
            ek = st[b].pop(f"ek{tp}")
            vb = st[b].pop(f"vb{tp}")
            for p in range(2):
                nc.tensor.matmul(
                    pctx[:, p, :],
                    lhsT=ek[:, :, p * P:(p + 1) * P],
                    rhs=vb[:, :, p, :],
                    start=(tp == 0), stop=(tp == NTP - 1),
                    perf_mode=DR, skip_group_check=True,
                )

        def emit_q_chunk(b, j):
            # Q conv (fp8-DR) + exp for a [128, 1024] chunk, both k-blocks
            # interleaved as j = k*4 + jc
            k, jc = divmod(j, 4)
            if j == 0:
                st[b]["sqp"] = misc.tile([P, 2, 4], F32, name=f"sqp_{b}",
                                         tag="sqp")
                st[b]["expq"] = eqp.tile([P, 2, N], F8, name=f"expq_{b}",
                                         tag="expq")
            wk = slice(b * 256 + k * P, b * 256 + (k + 1) * P)
            pq = qp.tile([P, 1024], F32, name=f"pq_{b}_{j}", tag="pq")
            for h in range(2):
                s = slice(jc * 1024 + h * 512, jc * 1024 + (h + 1) * 512)
                nc.tensor.matmul(
                    pq[:, h * 512:(h + 1) * 512],
                    lhsT=wq8[:, :, wk],
                    rhs=xq8[b][:, :, s],
                    start=True, stop=True, perf_mode=DR,
                )
            nc.scalar.activation(
                st[b]["expq"][:, k, jc * 1024:(jc + 1) * 1024], pq, AF.Exp,
                accum_out=st[b]["sqp"][:, k, jc:jc + 1])

        def emit_pctx_evict(b):
            # free the single pctx bank for the other branch (scalar engine:
            # it is off the vector queue's critical path); den + raw ctx
            pctx = st[b].pop("pctx")
            denT = misc.tile([P, 2], F32, name=f"denT_{b}", tag="denT")
            for p in range(2):
                nc.vector.tensor_copy(denT[:, p:p + 1], pctx[:, p, 128:129])
            ctxs = misc.tile([P, 2, P], BF16, name=f"ctxs_{b}", tag="ctxs")
            for p in range(2):
                nc.vector.tensor_copy(ctxs[:, p, :], pctx[:, p, 0:P])
            st[b]["denT"] = denT
            st[b]["ctxs"] = ctxs

        def emit_ctxT(b):
            # PE transpose of ctx + bv fold; independent of fac/sumq
            ctxs = st[b]["ctxs"]
            st[b]["pmt"] = []
            for p in range(2):
                # one PSUM bank shared by the bf16 transpose target (bytes
                # 0:256) and the f32 M^T accumulator (bytes 512:1536)
                tiny = tinyp.tile([P, 384], F32, name=f"tiny_{b}_{p}",
                                  tag="tiny")
                ptr = tiny[:, 0:64].bitcast(BF16)
                pmt = tiny[:, 128:384]
                for hh in range(2):
                    s = slice(hh * HD, (hh + 1) * HD)
                    nc.tensor.transpose(ptr[s, s], ctxs[s, p, :][:, s],
                                        id_sb[s, s])
                ctxT = misc.tile([P, P], BF16, name=f"ctxT_{b}_{p}", tag="ctxT")
                for hh in range(2):
                    s = slice(hh * HD, (hh + 1) * HD)
                    nc.vector.tensor_scalar(
                        ctxT[s, s], ptr[s, s],
                        bp_sb[s, 4 + b * 2 + p:5 + b * 2 + p], None, OP.add)
                wc = (2 * b + p) * 256
                for hh in range(2):
                    s = slice(hh * HD, (hh + 1) * HD)
                    nc.tensor.matmul(
                        pmt[s, :], lhsT=ctxT[s, s], rhs=wpt[s, wc:wc + 256],
                        start=True, stop=True, skip_group_check=True,
                    )
                st[b]["pmt"].append(pmt)

        def emit_mt_final(b):
            # fac = 2^22 / (den_k * sum_q); only this part joins on sumq
            sq2 = misc.tile([P, 2], F32, name=f"sq2_{b}", tag="sq2")
            for k in range(2):
                nc.vector.reduce_sum(sq2[:, k:k + 1], st[b]["sqp"][:, k, :],
                                     axis=AX.X)
            fde = misc.tile([P, 2], F32, name=f"fde_{b}", tag="fde")
            nc.vector.scalar_tensor_tensor(fde, st[b]["denT"], ISMT, sq2,
                                           OP.mult, OP.mult)
            fac = misc.tile([P, 2], F32, name=f"fac_{b}", tag="fac")
            nc.vector.reciprocal(fac, fde)
            mt8 = misc.tile([P, 2, 256], F8, name=f"mt8_{b}", tag="mt8")
            for p in range(2):
                nc.vector.tensor_scalar(mt8[:, p, :], st[b]["pmt"][p],
                                        fac[:, p:p + 1], None, OP.mult)
            st[b]["mt8"] = mt8

        def emit_inter_chunk(b, j, scalar_evict=False):
            # inter matmul + fused (descale + residual + BN affine) eviction,
            # streaming straight to the output DMA.  j = k*4 + jc
            # scalar_evict: residual accumulated in psum by a PE identity
            # matmul of the 1/spi-prescaled base, evicted by the scalar
            # engine (activation Copy * spi) — used where vector is the
            # serial tail and scalar idles.
            k, jc = divmod(j, 4)
            mt8 = st[b]["mt8"]
            expq = st[b]["expq"]
            spi = bp_sb[:, 2 * b + k:2 * b + k + 1]
            stage = stgp.tile([P, 1024], F32, name=f"stage_{b}_{j}",
                              tag="stage")
            for h in range(2):
                j0 = jc * 1024 + h * 512
                pi = psr.tile([P, 512], F32, name=f"pi_{b}_{j}_{h}", tag="ps")
                nc.tensor.matmul(
                    pi,
                    lhsT=mt8[:, :, k * P:(k + 1) * P],
                    rhs=expq[:, :, j0:j0 + 512],
                    start=True, stop=not scalar_evict, perf_mode=DR,
                )
                if scalar_evict:
                    nc.tensor.matmul(
                        pi, lhsT=id_sb, rhs=xqb2[b][k][:, j0:j0 + 512],
                        start=False, stop=True, skip_group_check=True,
                    )
                    nc.scalar.activation(
                        stage[:, h * 512:(h + 1) * 512], pi, AF.Copy,
                        scale=spi)
                else:
                    nc.vector.scalar_tensor_tensor(
                        stage[:, h * 512:(h + 1) * 512], pi, spi,
                        xqb[b][k][:, j0:j0 + 512], OP.mult, OP.add)
                nc.sync.dma_start(
                    out=out_d[b][k * P:(k + 1) * P, j0:j0 + 512],
                    in_=stage[:, h * 512:(h + 1) * 512])

        # ---- schedule ----
        # branch-0 KV (ctx lagging two pairs)
        for tp in range(NTP):
            emit_kv_pair(0, tp)
            if tp >= 2:
                emit_ctx(0, tp - 2)
        emit_ctx(0, NTP - 2)
        emit_ctx(0, NTP - 1)
        emit_pctx_evict(0)
        # branch-0 Q interleaved with branch-1 KV
        for j in range(8):
            emit_q_chunk(0, j)
            for tp in (2 * j, 2 * j + 1):
                emit_kv_pair(1, tp)
                if tp >= 2:
                    emit_ctx(1, tp - 2)
        emit_ctx(1, NTP - 2)
        emit_ctx(1, NTP - 1)
        emit_ctxT(0)
        emit_mt_final(0)
        emit_pctx_evict(1)
        emit_ctxT(1)
        # branch-1 Q front-loaded 2:1 against branch-0 inter so fac(1) joins
        # early; then both inter phases overlap with evictions split across
        # the vector (STT) and PE+scalar (identity-accumulate + Copy) paths
        for jj in range(4):
            emit_q_chunk(1, 2 * jj)
            emit_q_chunk(1, 2 * jj + 1)
            emit_inter_chunk(0, jj)
        emit_mt_final(1)
        for j in range(4, 8):
            emit_inter_chunk(0, j, scalar_evict=(j >= 6))
            emit_inter_chunk(1, j - 4, scalar_evict=bool(j % 2))
        for j in range(4, 8):
            emit_inter_chunk(1, j, scalar_evict=bool(j % 2))

    nc.finalize()
    return nc


def _get_nc():
    if "nc" not in _CACHE:
        _CACHE["nc"] = _build()
    return _CACHE["nc"]


def _dr(x):
    # [256, n] -> DoubleRow interleave [128, 2, n]: slot s holds channel p+128s
    return np.ascontiguousarray(x.reshape(2, P, -1).transpose(1, 0, 2))


def _pack_host(inputs):
    import ml_dtypes
    f8 = ml_dtypes.float8_e4m3
    bf16 = ml_dtypes.bfloat16
    f32 = np.float32

    wts = []
    wqs = []
    wps = []
    for b in ("1", "2"):
        wk = np.asarray(inputs[f"w_k{b}"], f32).T * SW
        wv = np.asarray(inputs[f"w_v{b}"], f32).T * SW
        wts.append(_dr(np.concatenate([wk, wv], axis=1)))
        wqs.append(_dr(np.asarray(inputs[f"w_q{b}"], f32).T * SW))
        wpT = np.ascontiguousarray(np.asarray(inputs[f"w_proj{b}"], f32).T)
        wps.extend([wpT[0:P, :], wpT[P:C, :]])
    wt8 = np.concatenate(wts, axis=2).astype(f8)        # [128, 2, 1024]
    wq8 = np.concatenate(wqs, axis=2).astype(f8)        # [128, 2, 512]
    wpt = np.concatenate(wps, axis=1).astype(bf16)      # [128, 1024]
    id128 = np.eye(P, dtype=bf16)
    return (np.ascontiguousarray(wt8), np.ascontiguousarray(wq8),
            np.ascontiguousarray(wpt), np.ascontiguousarray(id128))


def kernel(rgb_low, rgb_high, dsm_low, dsm_high,
           w_q1, b_q1, w_k1, b_k1, w_v1, b_v1,
           w_q2, b_q2, w_k2, b_k2, w_v2, b_v2,
           w_proj1, b_proj1, w_proj2, b_proj2, gamma, beta,
           _trace=False):
    import ml_dtypes
    from concourse.bass_utils import run_bass_kernel_spmd
    f8 = ml_dtypes.float8_e4m3
    bf16 = ml_dtypes.bfloat16
    f32 = np.float32

    inputs = dict(w_q1=w_q1, w_k1=w_k1, w_v1=w_v1, w_proj1=w_proj1,
                  w_q2=w_q2, w_k2=w_k2, w_v2=w_v2, w_proj2=w_proj2)
    rl = np.asarray(rgb_low, dtype=f32)
    rh = np.asarray(rgb_high, dtype=f32)
    dl = np.asarray(dsm_low, dtype=f32)
    dh = np.asarray(dsm_high, dtype=f32)
    B = rl.shape[0]
    assert B == NCORES, f"expected batch {NCORES}, got {B}"

    wt8, wq8, wpt, id128 = _pack_host(inputs)
    g = np.asarray(gamma, f32)
    be = np.asarray(beta, f32)

    # host-side BN stats from x alone; |inter| ~ 1e-4 contributes ~2e-4
    # relative to the batch statistics, far below the accuracy gate.
    # b_proj shifts the mean exactly -> folded here.
    xq = [rl.reshape(B, C, N), rh.reshape(B, C, N)]
    xkv = [dh.reshape(B, C, N), dl.reshape(B, C, N)]
    bprj = [np.asarray(b_proj1, f32), np.asarray(b_proj2, f32)]
    bvs = [np.asarray(b_v1, f32), np.asarray(b_v2, f32)]
    s2 = []
    t2 = []
    for b in range(2):
        mu = xq[b].mean(axis=(0, 2)) + bprj[b]
        sd = np.sqrt(xq[b].var(axis=(0, 2)) + EPS)
        s2.append(g / sd)
        t2.append(be - mu * s2[b])

    # bp: [128,8] = (spi b0k0, b0k1, b1k0, b1k1, bv b0p0, b0p1, b1p0, b1p1)
    bp = np.stack([s2[0][:P] * ISMT, s2[0][P:] * ISMT,
                   s2[1][:P] * ISMT, s2[1][P:] * ISMT,
                   bvs[0][:P], bvs[0][P:], bvs[1][:P], bvs[1][P:]],
                  axis=1).astype(f32)

    in_maps = []
    for i in range(NCORES):
        m = {"wt8": wt8, "wq8": wq8, "wpt": wpt, "id128": id128,
             "bp": np.ascontiguousarray(bp)}
        for b in range(2):
            m[f"xq8_{b + 1}"] = _dr(xq[b][i] / SW).astype(f8)
            m[f"xkv8_{b + 1}"] = _dr(xkv[b][i] / SW).astype(f8)
            m[f"xqb_{b + 1}"] = np.ascontiguousarray(
                (xq[b][i] * s2[b][:, None] + t2[b][:, None]).astype(bf16))
        # residual base pre-scaled by 1/spi for the PE+scalar eviction path
        for b in range(2):
            m[f"xqb2_{b + 1}"] = np.ascontiguousarray(
                ((xq[b][i] + (t2[b] / s2[b])[:, None]) * SMT).astype(bf16))
        in_maps.append(m)

    res = run_bass_kernel_spmd(nc := _get_nc(), in_maps,
                               core_ids=list(range(NCORES)), trace=_trace)
    out_low = np.stack([res.results[i]["out1"].reshape(C, 64, 64)
                        for i in range(NCORES)])
    out_high = np.stack([res.results[i]["out2"].reshape(C, 64, 64)
                         for i in range(NCORES)])
    if _trace:
        _CACHE["last_results"] = res
    return (out_low, out_high, np.asarray(dsm_low), np.asarray(dsm_high))


# revision 31
# speedup vs baseline: 1.0845x; 1.0845x over previous
# Trainium2 Bass kernel for nn_CrossFrequencyInteraction.
#
# Reference computation (per batch item, two symmetric branches):
#   q = Wq @ x_q;  k = Wk @ x_kv;  v = Wv @ x_kv          (1x1 convs, C=256)
#   out = softmax_n(q) applied against ctx = softmax_n(k) @ v^T  (linear attn)
#   inter = Wp @ out;  x_q += inter
#   then training-mode BatchNorm over (B,H,W) on both updated rgb tensors.
#
# Sharding: data-parallel over batch (B=8 -> 1 item per core, 8 cores).
#
# Numerics / structure (validated against the fp32 reference, absmax-rel
# ~3e-3 vs the 2e-2 gate):
#   - The attention path contributes |inter| <~ 1e-4 against |x| ~ 5, so it
#     runs end-to-end in fp8 e4m3 with DoubleRow matmuls (K=256 per
#     instruction, 0.5 cyc/col).  Host uploads x/8 and weights*8 so q,k,v
#     keep their exact scale going into exp while both operands sit in fp8
#     normal range.
#   - |inter|'s contribution to the BN batch statistics is ~2e-4 relative,
#     far below tolerance, so BN stats are computed on the host from x
#     alone (full batch is available host-side).  b_proj shifts the mean
#     exactly and is folded into the host-side mean.  The BN affine then
#     folds into the upload (xqb' = x*g/sd + t) and into the inter
#     eviction scale, fusing residual + BN into one scalar_tensor_tensor
#     per chunk.  No collective, no bn_stats, no separate normalize pass.
#   - b_q, b_k shift softmax inputs by a per-row constant -> cancel exactly.
#   - b_v folded into ctx at eviction (exact when sum softmax_k = 1).
#   - softmax denominators (k and q) are folded into the tiny
#     M = Wp.blockdiag(ctx^T) matrix (computed via PE transpose of ctx),
#     so attention-out + projection become a single fp8 [256,256] @
#     [256,4096] DoubleRow matmul per branch.  M underflows fp8 and is
#     scaled by 2^22, descaled for free in the eviction STT.
#   - kT/vT are produced in transposed layout by using x as the stationary
#     matmul operand; softmax-k denominators come free from a ones column
#     appended to the vT stream tiles.
#
# Scheduling: engine queues execute in issue order, so branch-0 Q-conv
# chunks are interleaved with branch-1 KV pairs (and branch-0 inter with
# branch-1 Q) to keep the PE dense while psum evictions drain; ctx matmuls
# run one pair behind their evictions.

import numpy as np

C = 256
N = 4096
P = 128
NTP = 16          # pairs of 128-wide n-tiles (KV phase, fp8-DR over pairs)
NCORES = 8
HD = 64
EPS = 1e-5
SW = 8.0          # host weight/input scale around fp8
SMT = float(2.0 ** 22)   # fp8 scale for M^T
ISMT = float(2.0 ** -22)

_CACHE = {}


def _build():
    import concourse.bass as bass
    import concourse.bacc as bacc
    import concourse.tile as tile
    from concourse import mybir
    from contextlib import ExitStack

    F32 = mybir.dt.float32
    BF16 = mybir.dt.bfloat16
    F8 = mybir.dt.float8e4
    OP = mybir.AluOpType
    AF = mybir.ActivationFunctionType
    AX = mybir.AxisListType
    DR = mybir.MatmulPerfMode.DoubleRow

    nc = bacc.Bacc("TRN2", num_devices=NCORES)

    xq8_d = [nc.dram_tensor(n_, [P, 2, N], F8, kind="ExternalInput")
             for n_ in ("xq8_1", "xq8_2")]
    xkv8_d = [nc.dram_tensor(n_, [P, 2, N], F8, kind="ExternalInput")
              for n_ in ("xkv8_1", "xkv8_2")]
    # pre-normalized residual base: x*g/sd + (beta - (mu+b_proj)*g/sd)
    xqb_d = [nc.dram_tensor(n_, [C, N], BF16, kind="ExternalInput")
             for n_ in ("xqb_1", "xqb_2")]
    # wt8: [128,2,1024]: per branch 512 cols of [Wk^T|Wv^T]*8, DR-interleaved
    wt8_d = nc.dram_tensor("wt8", [P, 2, 1024], F8, kind="ExternalInput")
    # wq8: [128,2,512]: per branch 256 cols of Wq^T*8, DR-interleaved
    wq8_d = nc.dram_tensor("wq8", [P, 2, 512], F8, kind="ExternalInput")
    # wpt: [128, 4*256] bf16; block (2b+p) = Wp_b^T[p*128:(p+1)*128, :]
    wpt_d = nc.dram_tensor("wpt", [P, 1024], BF16, kind="ExternalInput")
    id_d = nc.dram_tensor("id128", [P, P], BF16, kind="ExternalInput")
    # bp: [128, 8]: (spi b0k0, b0k1, b1k0, b1k1, bv b0p0, b0p1, b1p0, b1p1)
    bp_d = nc.dram_tensor("bp", [P, 8], F32, kind="ExternalInput")
    # residual base pre-scaled by 1/spi (for the PE-accumulated eviction
    # path: psum += xqb2, evicted by the scalar engine * spi)
    xqb2_d = [nc.dram_tensor(n_, [C, N], BF16, kind="ExternalInput")
              for n_ in ("xqb2_1", "xqb2_2")]
    out_d = [nc.dram_tensor(n_, [C, N], F32, kind="ExternalOutput")
             for n_ in ("out1", "out2")]

    with ExitStack() as ctx:
        tc = ctx.enter_context(tile.TileContext(nc))
        const = ctx.enter_context(tc.tile_pool(name="const", bufs=1))
        xp = ctx.enter_context(tc.tile_pool(name="xp", bufs=1))
        eqp = ctx.enter_context(tc.tile_pool(name="eqp", bufs=2))
        ekp = ctx.enter_context(tc.tile_pool(name="ekp", bufs=4))
        misc = ctx.enter_context(tc.tile_pool(name="misc", bufs=2))
        stgp = ctx.enter_context(tc.tile_pool(name="stgp", bufs=6))
        psr = ctx.enter_context(tc.tile_pool(name="psr", bufs=3, space="PSUM"))
        qp = ctx.enter_context(tc.tile_pool(name="qp", bufs=1, space="PSUM"))
        ctxp = ctx.enter_context(tc.tile_pool(name="ctxp", bufs=1, space="PSUM"))
        tinyp = ctx.enter_context(tc.tile_pool(name="tinyp", bufs=2, space="PSUM"))

        # ---- loads: critical pieces first on sync; the rest on scalar ----
        wt8 = const.tile([P, 2, 1024], F8, name="wt8", tag="wt8")
        wq8 = const.tile([P, 2, 512], F8, name="wq8", tag="wq8")
        wpt = const.tile([P, 1024], BF16, name="wpt", tag="wpt")
        id_sb = const.tile([P, P], BF16, name="id", tag="id")
        bp_sb = const.tile([P, 8], F32, name="bp", tag="bp")
        xkv8 = [xp.tile([P, 2, N], F8, name=f"xkv8_{b}", tag=f"xkv8_{b}")
                for b in range(2)]
        xq8 = [xp.tile([P, 2, N], F8, name=f"xq8_{b}", tag=f"xq8_{b}")
               for b in range(2)]
        xqb = [[xp.tile([P, N], BF16, name=f"xqb{k}_{b}", tag=f"xqb{k}_{b}")
                for k in range(2)] for b in range(2)]

        nc.sync.dma_start(out=wt8, in_=wt8_d[:, :, :])
        nc.sync.dma_start(out=xkv8[0][:, :, 0:512], in_=xkv8_d[0][:, :, 0:512])
        nc.sync.dma_start(out=wq8, in_=wq8_d[:, :, :])
        for c0, c1 in ((512, 1536), (1536, 2816), (2816, N)):
            nc.sync.dma_start(out=xkv8[0][:, :, c0:c1],
                              in_=xkv8_d[0][:, :, c0:c1])
        for c0, c1 in ((0, 2048), (2048, N)):
            nc.sync.dma_start(out=xq8[0][:, :, c0:c1],
                              in_=xq8_d[0][:, :, c0:c1])
        for c0, c1 in ((0, 1024), (1024, 2048), (2048, N)):
            nc.sync.dma_start(out=xkv8[1][:, :, c0:c1],
                              in_=xkv8_d[1][:, :, c0:c1])
        for c0, c1 in ((0, 2048), (2048, N)):
            nc.sync.dma_start(out=xq8[1][:, :, c0:c1],
                              in_=xq8_d[1][:, :, c0:c1])
        nc.scalar.dma_start(out=wpt, in_=wpt_d[:, :])
        nc.scalar.dma_start(out=id_sb, in_=id_d[:, :])
        nc.scalar.dma_start(out=bp_sb, in_=bp_d[:, :])
        for b in range(2):
            for k in range(2):
                nc.sync.dma_start(out=xqb[b][k],
                                  in_=xqb_d[b][k * P:(k + 1) * P, :])
        xqb2 = [[xp.tile([P, N], BF16, name=f"xqb2_{k}_{b}",
                         tag=f"xqb2_{k}_{b}") for k in range(2)]
                for b in range(2)]
        for b in range(2):
            for k in range(2):
                nc.sync.dma_start(out=xqb2[b][k],
                                  in_=xqb2_d[b][k * P:(k + 1) * P, :])
        # vT ring buffers [128, tt, pair, 129]; col 128 is the ones column
        # that yields softmax-k denominators inside the ctx matmuls.
        vtb = []
        for i in range(3):
            t = const.tile([P, 2, 2, 129], F8, name=f"vtb{i}", tag=f"vtb{i}")
            nc.vector.memset(t[:, :, :, 128], 1.0)
            vtb.append(t)

        st = {0: {}, 1: {}}

        # ---- emission helpers (issue order == engine execution order) ----

        def emit_kv_pair(b, tp):
            # K conv, V conv (fp8-DR) for n-tiles (2tp, 2tp+1) + evictions
            w0 = b * 512
            pk = psr.tile([P, 512], F32, name=f"pk_{b}_{tp}", tag="ps")
            pv = psr.tile([P, 512], F32, name=f"pv_{b}_{tp}", tag="ps")
            for tt in range(2):
                s128 = slice(tp * 256 + tt * P, tp * 256 + (tt + 1) * P)
                nc.tensor.matmul(
                    pk[:, tt * 256:(tt + 1) * 256],
                    lhsT=xkv8[b][:, :, s128],
                    rhs=wt8[:, :, w0:w0 + 256],
                    start=True, stop=True, perf_mode=DR,
                )
            for tt in range(2):
                s128 = slice(tp * 256 + tt * P, tp * 256 + (tt + 1) * P)
                nc.tensor.matmul(
                    pv[:, tt * 256:(tt + 1) * 256],
                    lhsT=xkv8[b][:, :, s128],
                    rhs=wt8[:, :, w0 + 256:w0 + 512],
                    start=True, stop=True, perf_mode=DR,
                )
            ek = ekp.tile([P, 2, 256], F8, name=f"ek_{b}_{tp}", tag="ek")
            nc.scalar.activation(ek.rearrange("p s c -> p (s c)"), pk, AF.Exp)
            vb = vtb[tp % 3]
            nc.vector.tensor_copy(
                vb[:, :, :, 0:P],
                pv.rearrange("p (s g c) -> p s g c", s=2, g=2))
            st[b][f"ek{tp}"] = ek
            st[b][f"vb{tp}"] = vb

        def emit_ctx(b, tp):
            # fp8-DR ctx matmuls for pair tp (issued one pair behind)
            pctx = st[b].get("pctx")
            if pctx is None:
                pctx = ctxp.tile([P, 2, 129], F32, name=f"pctx_{b}", tag="pctx")
                st[b]["pctx"] = pctx
            ek = st[b].pop(f"ek{tp}")
            vb = st[b].pop(f"vb{tp}")
            for p in range(2):
                nc.tensor.matmul(
                    pctx[:, p, :],
                    lhsT=ek[:, :, p * P:(p + 1) * P],
                    rhs=vb[:, :, p, :],
                    start=(tp == 0), stop=(tp == NTP - 1),
                    perf_mode=DR, skip_group_check=True,
                )

        def emit_q_chunk(b, j):
            # Q conv (fp8-DR) + exp for a [128, 1024] chunk, both k-blocks
            # interleaved as j = k*4 + jc
            k, jc = divmod(j, 4)
            if j == 0:
                st[b]["sqp"] = misc.tile([P, 2, 4], F32, name=f"sqp_{b}",
                                         tag="sqp")
                st[b]["expq"] = eqp.tile([P, 2, N], F8, name=f"expq_{b}",
                                         tag="expq")
            wk = slice(b * 256 + k * P, b * 256 + (k + 1) * P)
            pq = qp.tile([P, 1024], F32, name=f"pq_{b}_{j}", tag="pq")
            for h in range(2):
                s = slice(jc * 1024 + h * 512, jc * 1024 + (h + 1) * 512)
                nc.tensor.matmul(
                    pq[:, h * 512:(h + 1) * 512],
                    lhsT=wq8[:, :, wk],
                    rhs=xq8[b][:, :, s],
                    start=True, stop=True, perf_mode=DR,
                )
            nc.scalar.activation(
                st[b]["expq"][:, k, jc * 1024:(jc + 1) * 1024], pq, AF.Exp,
                accum_out=st[b]["sqp"][:, k, jc:jc + 1])

        def emit_pctx_evict(b):
            # free the single pctx bank for the other branch (scalar engine:
            # it is off the vector queue's critical path); den + raw ctx
            pctx = st[b].pop("pctx")
            denT = misc.tile([P, 2], F32, name=f"denT_{b}", tag="denT")
            for p in range(2):
                nc.vector.tensor_copy(denT[:, p:p + 1], pctx[:, p, 128:129])
            ctxs = misc.tile([P, 2, P], BF16, name=f"ctxs_{b}", tag="ctxs")
            for p in range(2):
                nc.vector.tensor_copy(ctxs[:, p, :], pctx[:, p, 0:P])
            st[b]["denT"] = denT
            st[b]["ctxs"] = ctxs

        def emit_ctxT(b):
            # PE transpose of ctx + bv fold; independent of fac/sumq
            ctxs = st[b]["ctxs"]
            st[b]["pmt"] = []
            for p in range(2):
                # one PSUM bank shared by the bf16 transpose target (bytes
                # 0:256) and the f32 M^T accumulator (bytes 512:1536)
                tiny = tinyp.tile([P, 384], F32, name=f"tiny_{b}_{p}",
                                  tag="tiny")
                ptr = tiny[:, 0:64].bitcast(BF16)
                pmt = tiny[:, 128:384]
                for hh in range(2):
                    s = slice(hh * HD, (hh + 1) * HD)
                    nc.tensor.transpose(ptr[s, s], ctxs[s, p, :][:, s],
                                        id_sb[s, s])
                ctxT = misc.tile([P, P], BF16, name=f"ctxT_{b}_{p}", tag="ctxT")
                for hh in range(2):
                    s = slice(hh * HD, (hh + 1) * HD)
                    nc.vector.tensor_scalar(
                        ctxT[s, s], ptr[s, s],
                        bp_sb[s, 4 + b * 2 + p:5 + b * 2 + p], None, OP.add)
                wc = (2 * b + p) * 256
                for hh in range(2):
                    s = slice(hh * HD, (hh + 1) * HD)
                    nc.tensor.matmul(
                        pmt[s, :], lhsT=ctxT[s, s], rhs=wpt[s, wc:wc + 256],
                        start=True, stop=True, skip_group_check=True,
                    )
                st[b]["pmt"].append(pmt)

        def emit_mt_final(b):
            # fac = 2^22 / (den_k * sum_q); only this part joins on sumq
            sq2 = misc.tile([P, 2], F32, name=f"sq2_{b}", tag="sq2")
            for k in range(2):
                nc.vector.reduce_sum(sq2[:, k:k + 1], st[b]["sqp"][:, k, :],
                                     axis=AX.X)
            fde = misc.tile([P, 2], F32, name=f"fde_{b}", tag="fde")
            nc.vector.scalar_tensor_tensor(fde, st[b]["denT"], ISMT, sq2,
                                           OP.mult, OP.mult)
            fac = misc.tile([P, 2], F32, name=f"fac_{b}", tag="fac")
            nc.vector.reciprocal(fac, fde)
            mt8 = misc.tile([P, 2, 256], F8, name=f"mt8_{b}", tag="mt8")
            for p in range(2):
                nc.vector.tensor_scalar(mt8[:, p, :], st[b]["pmt"][p],
                                        fac[:, p:p + 1], None, OP.mult)
            st[b]["mt8"] = mt8

        def emit_inter_chunk(b, j, scalar_evict=False):
            # inter matmul + fused (descale + residual + BN affine) eviction,
            # streaming straight to the output DMA.  j = k*4 + jc
            # scalar_evict: residual accumulated in psum by a PE identity
            # matmul of the 1/spi-prescaled base, evicted by the scalar
            # engine (activation Copy * spi) — used where vector is the
            # serial tail and scalar idles.
            k, jc = divmod(j, 4)
            mt8 = st[b]["mt8"]
            expq = st[b]["expq"]
            spi = bp_sb[:, 2 * b + k:2 * b + k + 1]
            stage = stgp.tile([P, 1024], F32, name=f"stage_{b}_{j}",
                              tag="stage")
            for h in range(2):
                j0 = jc * 1024 + h * 512
                pi = psr.tile([P, 512], F32, name=f"pi_{b}_{j}_{h}", tag="ps")
                nc.tensor.matmul(
                    pi,
                    lhsT=mt8[:, :, k * P:(k + 1) * P],
                    rhs=expq[:, :, j0:j0 + 512],
                    start=True, stop=not scalar_evict, perf_mode=DR,
                )
                if scalar_evict:
                    nc.tensor.matmul(
                        pi, lhsT=id_sb, rhs=xqb2[b][k][:, j0:j0 + 512],
                        start=False, stop=True, skip_group_check=True,
                    )
                    nc.scalar.activation(
                        stage[:, h * 512:(h + 1) * 512], pi, AF.Copy,
                        scale=spi)
                else:
                    nc.vector.scalar_tensor_tensor(
                        stage[:, h * 512:(h + 1) * 512], pi, spi,
                        xqb[b][k][:, j0:j0 + 512], OP.mult, OP.add)
                nc.sync.dma_start(
                    out=out_d[b][k * P:(k + 1) * P, j0:j0 + 512],
                    in_=stage[:, h * 512:(h + 1) * 512])

        # ---- schedule ----
        # branch-0 KV (ctx lagging two pairs)
        for tp in range(NTP):
            emit_kv_pair(0, tp)
            if tp >= 2:
                emit_ctx(0, tp - 2)
        emit_ctx(0, NTP - 2)
        emit_ctx(0, NTP - 1)
        emit_pctx_evict(0)
        # branch-0 Q interleaved with branch-1 KV
        for j in range(8):
            emit_q_chunk(0, j)
            for tp in (2 * j, 2 * j + 1):
                emit_kv_pair(1, tp)
                if tp >= 2:
                    emit_ctx(1, tp - 2)
        emit_ctx(1, NTP - 2)
        emit_ctx(1, NTP - 1)
        emit_ctxT(0)
        emit_mt_final(0)
        emit_pctx_evict(1)
        emit_ctxT(1)
        # branch-0 inter interleaved with branch-1 Q; branch-1 inter closes
        # with evictions split across the vector (STT) and PE+scalar
        # (identity-accumulate + Copy) paths
        for j in range(8):
            emit_q_chunk(1, j)
            emit_inter_chunk(0, j)
        emit_mt_final(1)
        for j in range(8):
            emit_inter_chunk(1, j, scalar_evict=bool(j % 2))

    nc.finalize()
    return nc


def _get_nc():
    if "nc" not in _CACHE:
        _CACHE["nc"] = _build()
    return _CACHE["nc"]


def _dr(x):
    # [256, n] -> DoubleRow interleave [128, 2, n]: slot s holds channel p+128s
    return np.ascontiguousarray(x.reshape(2, P, -1).transpose(1, 0, 2))


def _pack_host(inputs):
    import ml_dtypes
    f8 = ml_dtypes.float8_e4m3
    bf16 = ml_dtypes.bfloat16
    f32 = np.float32

    wts = []
    wqs = []
    wps = []
    for b in ("1", "2"):
        wk = np.asarray(inputs[f"w_k{b}"], f32).T * SW
        wv = np.asarray(inputs[f"w_v{b}"], f32).T * SW
        wts.append(_dr(np.concatenate([wk, wv], axis=1)))
        wqs.append(_dr(np.asarray(inputs[f"w_q{b}"], f32).T * SW))
        wpT = np.ascontiguousarray(np.asarray(inputs[f"w_proj{b}"], f32).T)
        wps.extend([wpT[0:P, :], wpT[P:C, :]])
    wt8 = np.concatenate(wts, axis=2).astype(f8)        # [128, 2, 1024]
    wq8 = np.concatenate(wqs, axis=2).astype(f8)        # [128, 2, 512]
    wpt = np.concatenate(wps, axis=1).astype(bf16)      # [128, 1024]
    id128 = np.eye(P, dtype=bf16)
    return (np.ascontiguousarray(wt8), np.ascontiguousarray(wq8),
            np.ascontiguousarray(wpt), np.ascontiguousarray(id128))


def kernel(rgb_low, rgb_high, dsm_low, dsm_high,
           w_q1, b_q1, w_k1, b_k1, w_v1, b_v1,
           w_q2, b_q2, w_k2, b_k2, w_v2, b_v2,
           w_proj1, b_proj1, w_proj2, b_proj2, gamma, beta,
           _trace=False):
    import ml_dtypes
    from concourse.bass_utils import run_bass_kernel_spmd
    f8 = ml_dtypes.float8_e4m3
    bf16 = ml_dtypes.bfloat16
    f32 = np.float32

    inputs = dict(w_q1=w_q1, w_k1=w_k1, w_v1=w_v1, w_proj1=w_proj1,
                  w_q2=w_q2, w_k2=w_k2, w_v2=w_v2, w_proj2=w_proj2)
    rl = np.asarray(rgb_low, dtype=f32)
    rh = np.asarray(rgb_high, dtype=f32)
    dl = np.asarray(dsm_low, dtype=f32)
    dh = np.asarray(dsm_high, dtype=f32)
    B = rl.shape[0]
    assert B == NCORES, f"expected batch {NCORES}, got {B}"

    wt8, wq8, wpt, id128 = _pack_host(inputs)
    g = np.asarray(gamma, f32)
    be = np.asarray(beta, f32)

    # host-side BN stats from x alone; |inter| ~ 1e-4 contributes ~2e-4
    # relative to the batch statistics, far below the accuracy gate.
    # b_proj shifts the mean exactly -> folded here.
    xq = [rl.reshape(B, C, N), rh.reshape(B, C, N)]
    xkv = [dh.reshape(B, C, N), dl.reshape(B, C, N)]
    bprj = [np.asarray(b_proj1, f32), np.asarray(b_proj2, f32)]
    bvs = [np.asarray(b_v1, f32), np.asarray(b_v2, f32)]
    s2 = []
    t2 = []
    for b in range(2):
        mu = xq[b].mean(axis=(0, 2)) + bprj[b]
        sd = np.sqrt(xq[b].var(axis=(0, 2)) + EPS)
        s2.append(g / sd)
        t2.append(be - mu * s2[b])

    # bp: [128,8] = (spi b0k0, b0k1, b1k0, b1k1, bv b0p0, b0p1, b1p0, b1p1)
    bp = np.stack([s2[0][:P] * ISMT, s2[0][P:] * ISMT,
                   s2[1][:P] * ISMT, s2[1][P:] * ISMT,
                   bvs[0][:P], bvs[0][P:], bvs[1][:P], bvs[1][P:]],
                  axis=1).astype(f32)

    in_maps = []
    for i in range(NCORES):
        m = {"wt8": wt8, "wq8": wq8, "wpt": wpt, "id128": id128,
             "bp": np.ascontiguousarray(bp)}
        for b in range(2):
            m[f"xq8_{b + 1}"] = _dr(xq[b][i] / SW).astype(f8)
            m[f"xkv8_{b + 1}"] = _dr(xkv[b][i] / SW).astype(f8)
            m[f"xqb_{b + 1}"] = np.ascontiguousarray(
                (xq[b][i] * s2[b][:, None] + t2[b][:, None]).astype(bf16))
        # residual base pre-scaled by 1/spi for the PE+scalar eviction path
        for b in range(2):
            m[f"xqb2_{b + 1}"] = np.ascontiguousarray(
                ((xq[b][i] + (t2[b] / s2[b])[:, None]) * SMT).astype(bf16))
        in_maps.append(m)

    res = run_bass_kernel_spmd(nc := _get_nc(), in_maps,
                               core_ids=list(range(NCORES)), trace=_trace)
    out_low = np.stack([res.results[i]["out1"].reshape(C, 64, 64)
                        for i in range(NCORES)])
    out_high = np.stack([res.results[i]["out2"].reshape(C, 64, 64)
                         for i in range(NCORES)])
    if _trace:
        _CACHE["last_results"] = res
    return (out_low, out_high, np.asarray(dsm_low), np.asarray(dsm_high))
